# revision 5
# baseline (speedup 1.0000x reference)
"""Causal multi-head attention (B=4, S=2048, D=1024, H=16) on 8 Trainium2 cores.

Sharding: core c handles batch b = c//2 and head-half hh = c%2 (8 heads, 512
head-dims). QKV/out projections are tensor-parallel over the head dim;
attention is embarrassingly parallel over (b, head). The out-projection
partial products (rank-512 each) are summed pairwise on the host along with
the output bias.

On-device layout is fully transposed (d_model / head-dim on partitions,
sequence on the free axis) so every matmul contracts over the partition dim
with no on-chip transposes:
  Q^T = (Wq*scale)^T-tiles.T @ x^T     K^T likewise      V natural = x^T.T @ Wv
  S^T[k,q] = K_h @ Q_h^T               exp via ScalarE (no max subtraction:
                                       |scores| <~ 4, exp is safe in fp32/bf16)
  [O^T; l] = [V_h | 1]^T.T @ expS^T    (ones column makes PV emit softmax
                                       denominators for free)
  O_norm^T = O^T * (1/l)               Y^T-partial = Wo-tiles.T @ O_cat^T
Causality: fully-masked key-blocks are skipped; diagonal blocks multiply
expS by a sliced 0/1 triangular mask.
"""

import numpy as np
import ml_dtypes

B, S, D = 4, 2048, 1024
H = 16
HH = 8          # heads per core
DK = 64
HD = 512        # head dims per core
N_CORES = 8
SCALE = DK ** -0.5
PB = 128        # partition block
QB = 512        # query block (matmul free dim)
NQB = S // QB   # 4
NKB = S // PB   # 16
KD = D // PB    # 8
KO = HD // PB   # 4
MW = 384 + QB   # mask width: slice origin 384-128*t for t in 0..3

_COMPILED = None
LAST_RESULTS = None


def _build():
    from contextlib import ExitStack
    import concourse.bass as bass
    import concourse.tile as tile
    from concourse import bacc, mybir

    BF16 = mybir.dt.bfloat16
    F32 = mybir.dt.float32
    AF = mybir.ActivationFunctionType

    nc = bacc.Bacc("TRN2", target_bir_lowering=False, debug=False,
                   num_devices=N_CORES)

    xT_d = nc.dram_tensor("xT", [D, S], BF16, kind="ExternalInput")
    wq_d = nc.dram_tensor("wq", [D, HD], BF16, kind="ExternalInput")
    wk_d = nc.dram_tensor("wk", [D, HD], BF16, kind="ExternalInput")
    wv_d = nc.dram_tensor("wv", [D, HD], BF16, kind="ExternalInput")
    wo_d = nc.dram_tensor("wo", [HD, D], BF16, kind="ExternalInput")
    bq_d = nc.dram_tensor("bq", [PB, KO], F32, kind="ExternalInput")
    bk_d = nc.dram_tensor("bk", [PB, KO], F32, kind="ExternalInput")
    bv_d = nc.dram_tensor("bv", [1, HD], BF16, kind="ExternalInput")
    mk_d = nc.dram_tensor("mk", [PB, MW], BF16, kind="ExternalInput")
    on_d = nc.dram_tensor("on", [1, PB], BF16, kind="ExternalInput")
    yT_d = nc.dram_tensor("yT", [D, S], F32, kind="ExternalOutput")

    with tile.TileContext(nc) as tc, ExitStack() as ctx:
        persist = ctx.enter_context(tc.tile_pool(name="persist", bufs=1))
        work = ctx.enter_context(tc.tile_pool(name="work", bufs=4))
        psA = ctx.enter_context(tc.tile_pool(name="psA", bufs=3, space="PSUM"))
        psO = ctx.enter_context(tc.tile_pool(name="psO", bufs=2, space="PSUM"))

        xT = persist.tile([PB, KD, S], BF16)
        wq = persist.tile([PB, KD, HD], BF16)
        wk = persist.tile([PB, KD, HD], BF16)
        wv = persist.tile([PB, KD, HD], BF16)
        wo = persist.tile([PB, KO, D], BF16)
        bq = persist.tile([PB, KO], F32)
        bk = persist.tile([PB, KO], F32)
        bv = persist.tile([1, HD], BF16)
        mk = persist.tile([PB, MW], BF16)
        ones = persist.tile([1, PB], BF16)
        qT = persist.tile([PB, KO, S], BF16)
        kT = persist.tile([PB, KO, S], BF16)
        # per head: 64 V columns + 64 ones columns (PV then emits the softmax
        # denominator replicated on partitions 64..127 — no broadcast needed)
        v = persist.tile([PB, NKB, HH, 2 * DK], BF16)
        onorm = persist.tile([PB, KO, S], BF16)

        for k in range(KD):
            nc.sync.dma_start(xT[:, k, :], xT_d[k * PB:(k + 1) * PB, :])
            nc.sync.dma_start(wq[:, k, :], wq_d[k * PB:(k + 1) * PB, :])
            nc.sync.dma_start(wk[:, k, :], wk_d[k * PB:(k + 1) * PB, :])
            nc.sync.dma_start(wv[:, k, :], wv_d[k * PB:(k + 1) * PB, :])
        for k2 in range(KO):
            nc.sync.dma_start(wo[:, k2, :], wo_d[k2 * PB:(k2 + 1) * PB, :])
        nc.sync.dma_start(bq[:], bq_d[:])
        nc.sync.dma_start(bk[:], bk_d[:])
        nc.sync.dma_start(bv[:], bv_d[:])
        nc.sync.dma_start(mk[:], mk_d[:])
        nc.sync.dma_start(ones[:], on_d[:])
        nc.vector.memset(v[:, :, :, DK:2 * DK], 1.0)

        # Q^T and K^T projections: [dk-block m, q-block] tiles, contract d_model
        for m in range(KO):
            for qb in range(NQB):
                qs = slice(qb * QB, (qb + 1) * QB)
                accq = psA.tile([PB, QB], F32, tag="acc")
                for k in range(KD):
                    nc.tensor.matmul(accq[:], wq[:, k, m * PB:(m + 1) * PB],
                                     xT[:, k, qs], start=(k == 0),
                                     stop=(k == KD - 1))
                nc.scalar.activation(qT[:, m, qs], accq[:], AF.Identity,
                                     bias=bq[:, m:m + 1], scale=1.0)
                acck = psA.tile([PB, QB], F32, tag="acc")
                for k in range(KD):
                    nc.tensor.matmul(acck[:], wk[:, k, m * PB:(m + 1) * PB],
                                     xT[:, k, qs], start=(k == 0),
                                     stop=(k == KD - 1))
                nc.scalar.activation(kT[:, m, qs], acck[:], AF.Identity,
                                     bias=bk[:, m:m + 1], scale=1.0)

        # V projection in natural layout [keys, head-dims], bias via rank-1 matmul
        for kb in range(NKB):
            accv = psA.tile([PB, QB], F32, tag="acc")
            for k in range(KD):
                nc.tensor.matmul(accv[:], xT[:, k, kb * PB:(kb + 1) * PB],
                                 wv[:, k, :], start=(k == 0), stop=False)
            nc.tensor.matmul(accv[:], ones[0:1, :], bv[0:1, :],
                             start=False, stop=True)
            nc.vector.tensor_copy(v[:, kb, :, 0:DK],
                                  accv[:].rearrange("p (h d) -> p h d", h=HH))

        # Attention per (head, q-block)
        for h in range(HH):
            m, rb = h // 2, DK * (h % 2)
            for qb in range(NQB):
                qs = slice(qb * QB, (qb + 1) * QB)
                nkb = 4 * qb + 4
                o_acc = psO.tile([PB, QB], F32, tag="oacc")
                for kb in range(nkb):
                    s_ps = psA.tile([PB, QB], F32, tag="acc")
                    nc.tensor.matmul(
                        s_ps[:], kT[rb:rb + DK, m, kb * PB:(kb + 1) * PB],
                        qT[rb:rb + DK, m, qs], start=True, stop=True)
                    e_sb = work.tile([PB, QB], BF16, tag="exp")
                    nc.scalar.activation(e_sb[:], s_ps[:], AF.Exp)
                    if kb >= 4 * qb:
                        t = kb - 4 * qb
                        u0 = 384 - 128 * t
                        nc.vector.tensor_mul(e_sb[:], e_sb[:],
                                             mk[:, u0:u0 + QB])
                    nc.tensor.matmul(o_acc[:], v[:, kb, h, :],
                                     e_sb[:], start=(kb == 0),
                                     stop=(kb == nkb - 1))
                r_sb = work.tile([DK, QB], F32, tag="r")
                nc.vector.reciprocal(r_sb[:], o_acc[DK:2 * DK, :])
                nc.vector.tensor_mul(onorm[rb:rb + DK, m, qs],
                                     o_acc[0:DK, :], r_sb[:])

        # Out projection partial: Y^T = sum_k2 Wo[k2].T @ O_cat^T[k2]
        for mo in range(KD):
            for qb in range(NQB):
                qs = slice(qb * QB, (qb + 1) * QB)
                y_ps = psA.tile([PB, QB], F32, tag="acc")
                for k2 in range(KO):
                    nc.tensor.matmul(y_ps[:], wo[:, k2, mo * PB:(mo + 1) * PB],
                                     onorm[:, k2, qs], start=(k2 == 0),
                                     stop=(k2 == KO - 1))
                y_sb = work.tile([PB, QB], F32, tag="y")
                nc.vector.tensor_copy(y_sb[:], y_ps[:])
                nc.sync.dma_start(yT_d[mo * PB:(mo + 1) * PB, qs], y_sb[:])

    nc.compile()
    return nc


def _get_compiled():
    global _COMPILED
    if _COMPILED is None:
        _COMPILED = _build()
    return _COMPILED


def _make_in_maps(x, Wq, bq, Wk, bk, Wv, bv, Wo):
    bf16 = ml_dtypes.bfloat16
    f32 = np.float32

    # 0/1 triangular mask: mk[p, u] = 1 iff p <= u - 384
    p_idx = np.arange(PB)[:, None]
    u_idx = np.arange(MW)[None, :]
    mk = (p_idx <= u_idx - 384).astype(bf16)
    ones = np.ones((1, PB), dtype=bf16)

    in_maps = []
    for c in range(N_CORES):
        b, hh = c // 2, c % 2
        cs = slice(hh * HD, (hh + 1) * HD)
        in_maps.append({
            "xT": np.ascontiguousarray(x[b].T).astype(bf16),
            "wq": (Wq[:, cs] * SCALE).astype(bf16),
            "wk": np.ascontiguousarray(Wk[:, cs]).astype(bf16),
            "wv": np.ascontiguousarray(Wv[:, cs]).astype(bf16),
            "wo": np.ascontiguousarray(Wo[cs, :]).astype(bf16),
            "bq": np.ascontiguousarray(
                (bq[cs] * SCALE).astype(f32).reshape(KO, PB).T),
            "bk": np.ascontiguousarray(
                bk[cs].astype(f32).reshape(KO, PB).T),
            "bv": bv[cs].astype(bf16).reshape(1, HD),
            "mk": mk,
            "on": ones,
        })
    return in_maps


def _reference_fallback(x, mask, Wq, bq, Wk, bk, Wv, bv, Wo, bo):
    out = np.empty((B, S, D), dtype=np.float32)
    for b in range(B):
        q = (x[b] @ Wq + bq).reshape(S, H, DK).transpose(1, 0, 2)
        k = (x[b] @ Wk + bk).reshape(S, H, DK).transpose(1, 0, 2)
        vv = (x[b] @ Wv + bv).reshape(S, H, DK).transpose(1, 0, 2)
        o = np.empty((H, S, DK), dtype=np.float32)
        for hi in range(H):
            s = (q[hi] @ k[hi].T) * SCALE
            s = np.where(mask[b], -1e9, s)
            s = s - s.max(axis=-1, keepdims=True)
            e = np.exp(s)
            p = e / e.sum(axis=-1, keepdims=True)
            o[hi] = p @ vv[hi]
        out[b] = o.transpose(1, 0, 2).reshape(S, D) @ Wo + bo
    return out


def kernel(x, mask, Wq, bq, Wk, bk, Wv, bv, Wo, bo, **kwargs):
    global LAST_RESULTS
    import os

    x = np.asarray(x, dtype=np.float32)
    mask = np.asarray(mask)

    causal = np.triu(np.ones((S, S), dtype=bool), k=1)
    if not all(np.array_equal(mask[b], causal) for b in range(B)):
        return _reference_fallback(np.asarray(x), mask, np.asarray(Wq),
                                   np.asarray(bq), np.asarray(Wk),
                                   np.asarray(bk), np.asarray(Wv),
                                   np.asarray(bv), np.asarray(Wo),
                                   np.asarray(bo))

    from concourse.bass_utils import run_bass_kernel_spmd

    nc = _get_compiled()
    in_maps = _make_in_maps(x, np.asarray(Wq), np.asarray(bq), np.asarray(Wk),
                            np.asarray(bk), np.asarray(Wv), np.asarray(bv),
                            np.asarray(Wo))
    trace = bool(int(os.environ.get("KERNEL_PROFILE", "0")))
    res = run_bass_kernel_spmd(nc, in_maps, list(range(N_CORES)), trace=trace)
    LAST_RESULTS = res

    bo32 = np.asarray(bo, dtype=np.float32)
    out = np.empty((B, S, D), dtype=np.float32)
    for b in range(B):
        acc = res.results[2 * b]["yT"] + res.results[2 * b + 1]["yT"]
        out[b] = acc.T + bo32
    return out


# revision 8
# speedup vs baseline: 1.4129x; 1.4129x over previous
"""Causal multi-head attention (B=4, S=2048, D=1024, H=16) on 8 Trainium2 cores.

Sharding: core c handles batch b = c//2 and head-half hh = c%2 (8 heads, 512
head-dims). QKV/out projections are tensor-parallel over the head dim;
attention is embarrassingly parallel over (b, head). The out-projection
partial products (rank-512 each) are summed pairwise on the host along with
the output bias.

On-device layout is fully transposed (d_model / head-dim on partitions,
sequence on the free axis) so every matmul contracts over the partition dim
with no on-chip transposes:
  Q^T = (Wq*scale)-tiles.T @ x^T      K^T likewise     V natural = x^T.T @ Wv
  S^T[k,q] = K_h @ Q_h^T              exp on ScalarE (no max subtraction:
                                      |scores| <~ 5, exp is safe in fp32)
  [O^T; l..l] = [V_h | 1s].T @ expS^T (64 ones columns in the stationary make
                                      PV emit the softmax denominator
                                      replicated on partitions 64..127)
  O_norm^T = O^T * (1/l)              Y^T-partial = Wo-tiles.T @ O_cat^T
Causality: fully-masked key-blocks are skipped; diagonal blocks compute only
the valid column range and apply one 128x128 triangular mask multiply.
Head pairs (even/odd) interleave so K=64 score matmuls pack into disjoint
PE row-groups and run concurrently.
"""

import numpy as np
import ml_dtypes

B, S, D = 4, 2048, 1024
H = 16
HH = 8          # heads per core
DK = 64
HD = 512        # head dims per core
N_CORES = 8
SCALE = DK ** -0.5
PB = 128        # partition block
QB = 512        # query block (matmul free dim)
NQB = S // QB   # 4
NKB = S // PB   # 16
KD = D // PB    # 8
KO = HD // PB   # 4

_COMPILED = None
LAST_RESULTS = None


def _build():
    from contextlib import ExitStack
    import concourse.bass as bass
    import concourse.tile as tile
    from concourse import bacc, mybir

    BF16 = mybir.dt.bfloat16
    F32 = mybir.dt.float32
    AF = mybir.ActivationFunctionType

    nc = bacc.Bacc("TRN2", target_bir_lowering=False, debug=False,
                   num_devices=N_CORES)

    xT_d = nc.dram_tensor("xT", [D, S], BF16, kind="ExternalInput")
    wq_d = nc.dram_tensor("wq", [D, HD], BF16, kind="ExternalInput")
    wk_d = nc.dram_tensor("wk", [D, HD], BF16, kind="ExternalInput")
    wv_d = nc.dram_tensor("wv", [D, HD], BF16, kind="ExternalInput")
    wo_d = nc.dram_tensor("wo", [HD, D], BF16, kind="ExternalInput")
    bq_d = nc.dram_tensor("bq", [PB, KO], F32, kind="ExternalInput")
    bk_d = nc.dram_tensor("bk", [PB, KO], F32, kind="ExternalInput")
    bvb_d = nc.dram_tensor("bvb", [PB, HD], BF16, kind="ExternalInput")
    mk_d = nc.dram_tensor("mk", [PB, PB], BF16, kind="ExternalInput")
    yT_d = nc.dram_tensor("yT", [D, S], F32, kind="ExternalOutput")

    with tile.TileContext(nc) as tc, ExitStack() as ctx:
        persist = ctx.enter_context(tc.tile_pool(name="persist", bufs=1))
        work = ctx.enter_context(tc.tile_pool(name="work", bufs=8))
        nrm = ctx.enter_context(tc.tile_pool(name="nrm", bufs=4))
        psA = ctx.enter_context(tc.tile_pool(name="psA", bufs=4, space="PSUM"))
        psO = ctx.enter_context(tc.tile_pool(name="psO", bufs=4, space="PSUM"))

        xT = [persist.tile([PB, S], BF16, name=f"xT{k}") for k in range(KD)]
        wq = [persist.tile([PB, HD], BF16, name=f"wq{k}") for k in range(KD)]
        wk = [persist.tile([PB, HD], BF16, name=f"wk{k}") for k in range(KD)]
        wv = [persist.tile([PB, HD], BF16, name=f"wv{k}") for k in range(KD)]
        wo = [persist.tile([PB, D], BF16, name=f"wo{k}") for k in range(KO)]
        bq = persist.tile([PB, KO], F32)
        bk = persist.tile([PB, KO], F32)
        bvb = persist.tile([PB, HD], BF16)
        mk = persist.tile([PB, PB], BF16)
        qT = [persist.tile([PB, S], BF16, name=f"qT{k}") for k in range(KO)]
        kT = [persist.tile([PB, S], BF16, name=f"kT{k}") for k in range(KO)]
        # per key-block: 8 heads x (64 ones columns + 64 V columns).
        # Ones first so the PV matmul puts the softmax denominator on
        # partitions 0..63 (reciprocal_approx_fast requires base partition 0).
        v = [persist.tile([PB, HH, 2 * DK], BF16, name=f"v{k}") for k in range(NKB)]
        onorm = [persist.tile([PB, S], BF16, name=f"onorm{k}") for k in range(KO)]

        for k in range(KD):
            nc.sync.dma_start(xT[k][:], xT_d[k * PB:(k + 1) * PB, :])
            nc.sync.dma_start(wq[k][:], wq_d[k * PB:(k + 1) * PB, :])
            nc.sync.dma_start(wk[k][:], wk_d[k * PB:(k + 1) * PB, :])
            nc.sync.dma_start(wv[k][:], wv_d[k * PB:(k + 1) * PB, :])
        for k2 in range(KO):
            nc.sync.dma_start(wo[k2][:], wo_d[k2 * PB:(k2 + 1) * PB, :])
        nc.sync.dma_start(bq[:], bq_d[:])
        nc.sync.dma_start(bk[:], bk_d[:])
        nc.sync.dma_start(bvb[:], bvb_d[:])
        nc.sync.dma_start(mk[:], mk_d[:])
        for kb in range(NKB):
            nc.vector.memset(v[kb][:, :, 0:DK], 1.0)

        # V projection first (attention needs it for every head)
        for kb in range(NKB):
            accv = psA.tile([PB, QB], F32, tag="acc")
            for k in range(KD):
                nc.tensor.matmul(accv[:], xT[k][:, kb * PB:(kb + 1) * PB],
                                 wv[k][:], start=(k == 0), stop=(k == KD - 1))
            nc.vector.tensor_add(v[kb][:, :, DK:2 * DK],
                                 accv[:].rearrange("p (h d) -> p h d", h=HH),
                                 bvb[:].rearrange("p (h d) -> p h d", h=HH))

        def qk_proj(m):
            for qb in range(NQB):
                qs = slice(qb * QB, (qb + 1) * QB)
                accq = psA.tile([PB, QB], F32, tag="acc")
                for k in range(KD):
                    nc.tensor.matmul(accq[:], wq[k][:, m * PB:(m + 1) * PB],
                                     xT[k][:, qs], start=(k == 0),
                                     stop=(k == KD - 1))
                nc.vector.tensor_scalar_add(qT[m][:, qs], accq[:],
                                            bq[:, m:m + 1])
                acck = psA.tile([PB, QB], F32, tag="acc")
                for k in range(KD):
                    nc.tensor.matmul(acck[:], wk[k][:, m * PB:(m + 1) * PB],
                                     xT[k][:, qs], start=(k == 0),
                                     stop=(k == KD - 1))
                nc.vector.tensor_scalar_add(kT[m][:, qs], acck[:],
                                            bk[:, m:m + 1])

        def attention(hp, qb):
            # head pair 2*hp (rows 0:64) + 2*hp+1 (rows 64:128), query block qb
            m = hp
            qs = slice(qb * QB, (qb + 1) * QB)
            nkb = 4 * qb + 4
            o_accs = [psO.tile([PB, QB], F32, tag="oacc", name=f"oacc{i}") for i in range(2)]
            for kb in range(nkb):
                t = kb - 4 * qb
                c0 = 0 if t < 0 else 128 * t
                cs = slice(qb * QB + c0, (qb + 1) * QB)
                es = []
                for i, rb in enumerate((0, DK)):
                    h = 2 * hp + i
                    s_ps = psA.tile([PB, QB], F32, tag="acc")
                    nc.tensor.matmul(
                        s_ps[:, c0:QB],
                        kT[m][rb:rb + DK, kb * PB:(kb + 1) * PB],
                        qT[m][rb:rb + DK, cs], start=True, stop=True)
                    e_sb = work.tile([PB, QB], BF16, tag="exp")
                    nc.scalar.activation(e_sb[:, c0:QB], s_ps[:, c0:QB], AF.Exp)
                    if t >= 0:
                        nc.vector.tensor_mul(e_sb[:, c0:c0 + PB],
                                             e_sb[:, c0:c0 + PB], mk[:])
                    es.append(e_sb)
                for i in range(2):
                    h = 2 * hp + i
                    nc.tensor.matmul(o_accs[i][:, c0:QB],
                                     v[kb][:, h, :], es[i][:, c0:QB],
                                     start=(kb == 0), stop=(kb == nkb - 1),
                                     skip_group_check=True)
            for i, rb in enumerate((0, DK)):
                r_sb = nrm.tile([DK, QB], F32, tag="r")
                nc.vector.reciprocal_approx_fast(r_sb[:], o_accs[i][0:DK, :])
                nc.vector.tensor_mul(onorm[m][rb:rb + DK, qs],
                                     o_accs[i][DK:2 * DK, :], r_sb[:])

        # Interleave projections with attention so PE stays dense while the
        # ScalarE exp stream runs
        qk_proj(0)
        for hp in range(KO):
            for qb in range(NQB):
                attention(hp, qb)
            if hp + 1 < KO:
                qk_proj(hp + 1)

        # Out projection partial: Y^T = sum_k2 Wo[k2].T @ O_cat^T[k2]
        for mo in range(KD):
            for qb in range(NQB):
                qs = slice(qb * QB, (qb + 1) * QB)
                y_ps = psA.tile([PB, QB], F32, tag="acc")
                for k2 in range(KO):
                    nc.tensor.matmul(y_ps[:], wo[k2][:, mo * PB:(mo + 1) * PB],
                                     onorm[k2][:, qs], start=(k2 == 0),
                                     stop=(k2 == KO - 1))
                y_sb = nrm.tile([PB, QB], F32, tag="y")
                nc.vector.tensor_copy(y_sb[:], y_ps[:])
                nc.sync.dma_start(yT_d[mo * PB:(mo + 1) * PB, qs], y_sb[:])

    nc.compile()
    return nc


def _get_compiled():
    global _COMPILED
    if _COMPILED is None:
        _COMPILED = _build()
    return _COMPILED


def _make_in_maps(x, Wq, bq, Wk, bk, Wv, bv, Wo):
    bf16 = ml_dtypes.bfloat16
    f32 = np.float32

    # inclusive lower-triangular mask for diagonal 128x128 blocks
    p_idx = np.arange(PB)[:, None]
    c_idx = np.arange(PB)[None, :]
    mk = (p_idx <= c_idx).astype(bf16)

    in_maps = []
    for c in range(N_CORES):
        b, hh = c // 2, c % 2
        cs = slice(hh * HD, (hh + 1) * HD)
        in_maps.append({
            "xT": np.ascontiguousarray(x[b].T).astype(bf16),
            "wq": (Wq[:, cs] * SCALE).astype(bf16),
            "wk": np.ascontiguousarray(Wk[:, cs]).astype(bf16),
            "wv": np.ascontiguousarray(Wv[:, cs]).astype(bf16),
            "wo": np.ascontiguousarray(Wo[cs, :]).astype(bf16),
            "bq": np.ascontiguousarray(
                (bq[cs] * SCALE).astype(f32).reshape(KO, PB).T),
            "bk": np.ascontiguousarray(
                bk[cs].astype(f32).reshape(KO, PB).T),
            "bvb": np.broadcast_to(bv[cs].astype(bf16), (PB, HD)).copy(),
            "mk": mk,
        })
    return in_maps


def _reference_fallback(x, mask, Wq, bq, Wk, bk, Wv, bv, Wo, bo):
    out = np.empty((B, S, D), dtype=np.float32)
    for b in range(B):
        q = (x[b] @ Wq + bq).reshape(S, H, DK).transpose(1, 0, 2)
        k = (x[b] @ Wk + bk).reshape(S, H, DK).transpose(1, 0, 2)
        vv = (x[b] @ Wv + bv).reshape(S, H, DK).transpose(1, 0, 2)
        o = np.empty((H, S, DK), dtype=np.float32)
        for hi in range(H):
            s = (q[hi] @ k[hi].T) * SCALE
            s = np.where(mask[b], -1e9, s)
            s = s - s.max(axis=-1, keepdims=True)
            e = np.exp(s)
            p = e / e.sum(axis=-1, keepdims=True)
            o[hi] = p @ vv[hi]
        out[b] = o.transpose(1, 0, 2).reshape(S, D) @ Wo + bo
    return out


def kernel(x, mask, Wq, bq, Wk, bk, Wv, bv, Wo, bo, **kwargs):
    global LAST_RESULTS
    import os

    x = np.asarray(x, dtype=np.float32)
    mask = np.asarray(mask)

    causal = np.triu(np.ones((S, S), dtype=bool), k=1)
    if not all(np.array_equal(mask[b], causal) for b in range(B)):
        return _reference_fallback(np.asarray(x), mask, np.asarray(Wq),
                                   np.asarray(bq), np.asarray(Wk),
                                   np.asarray(bk), np.asarray(Wv),
                                   np.asarray(bv), np.asarray(Wo),
                                   np.asarray(bo))

    from concourse.bass_utils import run_bass_kernel_spmd

    nc = _get_compiled()
    in_maps = _make_in_maps(x, np.asarray(Wq), np.asarray(bq), np.asarray(Wk),
                            np.asarray(bk), np.asarray(Wv), np.asarray(bv),
                            np.asarray(Wo))
    trace = bool(int(os.environ.get("KERNEL_PROFILE", "0")))
    res = run_bass_kernel_spmd(nc, in_maps, list(range(N_CORES)), trace=trace)
    LAST_RESULTS = res

    bo32 = np.asarray(bo, dtype=np.float32)
    out = np.empty((B, S, D), dtype=np.float32)
    for b in range(B):
        acc = res.results[2 * b]["yT"] + res.results[2 * b + 1]["yT"]
        out[b] = acc.T + bo32
    return out
